# revision 20
# baseline (speedup 1.0000x reference)
"""ConvGRU Trainium2 kernel.

video [B=2, T=16, C=128, H=64, W=64] f32; 1x1-conv GRU over T.
Sharding: data-parallel over (B x H/16) -> 8 cores, each core owns
P = 16*64 = 1024 pixels for all T; weights replicated.

Per core, per timestep (pixels on the free dim, channels on partitions):
    zr_pre = [Wzx@x + Wzh@h | Wrx@x + Wrh@h]      (PE, fp16 in / fp32 psum)
    z = sigmoid(zr_pre[:P] + bz); r = sigmoid(zr_pre[P:] + br)   (ACT)
    rh = r * h                                     (DVE)
    c = tanh(Whx@x + Whh@rh + bh)                  (PE + ACT)
    h' = u + v,  u = zbar*h,  v = z*c,  zbar = sigmoid(-pre_z)

G=2 pixel groups form two independent recurrence chains that
interleave on the engines.  The Scalar (ACT) engine is the pacing
resource: 6 sigmoid/tanh ops x ~690ns = 4.13us/step of streaming.
The remaining slack is the serial tail between the last tanh and the
next step's first r-sigmoid.  Structure choices that close it:

  - The next step's r-gate close is DISTRIBUTED over h' = u + v:
        pre_r(t+1) += Wrh@u(t)   (issues mid-step, u is ready early)
        pre_r(t+1) += Wrh@v(t)   (right after v -- the h' add leaves
                                  the sigmoid critical path entirely)
    The z-gate close stays a single Wzh@h' (zbar sits early in the
    next step's ACT stream, so it has slack).
  - Each group's zbar runs immediately after its own r-sigmoid, so
    that group's next-step zr openers (WAR on the single-buffered zr
    PSUM tile) clear the PE FIFO long before the r-closes arrive.
  - DVE tail is group-major (u,z,v,add per group): the first group's
    v/add never queue behind the second group's u/z, whose zbar lands
    later on the ACT stream.
  - t=0 shortcut: h0 == 0, so closers, r-sigmoid and rh are skipped.
  - fp16 everywhere: bf16 measures uniformly slower on this stack
    (ACTIVATE 687->823ns, TT 423->508ns); fp16 matmuls already
    pipeline at the 216ns/MM N=512 roofline.
  - DMA traffic is split across the two fast queues (sync HWDGE +
    gpsimd SWDGE): per step one x prefetch (t+2 ahead) rides sync and
    the two h' output stores ride gpsimd.  The last step's outputs
    split across the scalar+sync queues instead (the ACT queue is
    idle by then and the gpsimd dge_drain would add ~3.6us of tail),
    with the final h'-adds chunked so each half's store dispatches as
    soon as that half is summed.
  - Startup: weights live in TWO tiles (Wzx|Whx|biases vs the rest)
    and x0/x1 in per-group half tiles, so t0's matmuls gate only on
    their own DMAs (dependency tracking is tile-granular).  Biases
    ride as 4 fp16 columns inside the first weight DMA (a separate
    [C,4] f32 tensor costs ~0.9us of 16-byte-packet DMA).  Queues:
    sync carries x0a/w-mid/x1a/x2, gpsimd carries w-lo/x0b/x1b, and
    the scalar queue carries NOTHING early -- a scalar dispatch next
    to the warmup sigmoid makes the compiler emit a second 1283ns
    ACT_TABLE_LOAD.
  - PE warmup matmuls run against a memset tile (no weight-DMA
    dependency); the warmup sigmoid follows the FIRST of them so the
    activation-table load finishes during the DMA ramp.  Ramp-phase
    matmuls run ~3x slow (pstate ramp), which keeps t0/t1 at ~10.5
    and ~15.9us despite data landing by ~12us.

Measured: 84.3-84.7us (3-run spread) vs 85.7-86.3us for the prior
best; steady-state period ~4210ns/step vs the 4128ns ACT-streaming
floor.  Run-to-run the shared-chip clock varies up to ~1.2x, so
single measurements of this kernel family overlap; structure is
compared via clock-normalized traces (ACTIVATE=687ns at full clock).

Numerics: fp16 matmul inputs/gates/state/biases, fp32 PSUM accum.
Failed directions (measured): merging z|r sigmoids into [C,1024]
ACTIVATEs needs per-half PSUM bias pre-fills that no engine can
afford (DVE tensor ops are 424ns, fills 833-1467ns, and GPSIMD has
no PSUM port); DVE at 3.96us/step secretly co-paces with ACT at
4.13, so any design that adds DVE work loses.
"""

import os
import sys

import numpy as np

B, T, C, H, W = 2, 16, 128, 64, 64
NCORES = 8
HQ = H // 4          # 16 rows of H per core (4 H-slices x 2 batches = 8 cores)
P = HQ * W           # 1024 pixels per core
G = 2                # pixel groups per step (independent recurrence chains)
PG = P // G          # 512 pixels per group

_PROG = None


def _ensure_paths():
    for p in ("/opt/trn_rl_repo",):
        if p not in sys.path and os.path.isdir(p):
            sys.path.append(p)


def _build():
    _ensure_paths()
    import concourse.bacc as bacc
    import concourse.tile as tile
    from concourse import mybir

    f32 = mybir.dt.float32
    f16 = mybir.dt.float16
    AF = mybir.ActivationFunctionType

    nc = bacc.Bacc(
        "TRN2", target_bir_lowering=False, debug=False, num_devices=NCORES
    )
    x_dram = nc.dram_tensor("x_seq", [T, C, P], f16, kind="ExternalInput")
    # wmats = [Wzx.T | Whx.T | bz,br,bh,-bz | Wrx.T | Wzh.T | Wrh.T | Whh.T]
    # biases ride as 4 fp16 columns inside the first weight DMA -- a
    # separate [C,4] tensor costs ~0.9us of 16B-packet DMA at startup
    w_dram = nc.dram_tensor("wmats", [C, 6 * C + 4], f16, kind="ExternalInput")
    o_dram = nc.dram_tensor("out_seq", [T, C, P], f16, kind="ExternalOutput")

    x_ap = x_dram.ap()
    w_ap = w_dram.ap()
    o_ap = o_dram.ap()

    # weight order in wmats: x-side first so its DMA can land first
    WZX, WHX, WRX, WZH, WRH, WHH = range(6)

    with tile.TileContext(nc) as tc:
        with (
            tc.tile_pool(name="consts", bufs=1) as consts,
            tc.tile_pool(name="xin", bufs=4) as xpool,
            tc.tile_pool(name="state", bufs=4) as spool,
            tc.tile_pool(name="work", bufs=3) as wk,
            tc.tile_pool(name="ps", bufs=1, space="PSUM") as ps,
        ):
            # weights live in two tiles so t0's matmuls only gate on the
            # (Wzx|Whx|biases) DMA, not on the whole weight load; likewise
            # x0/x1 are per-group half tiles so each group's openers gate
            # on their own half.
            wt_lo = consts.tile([C, 2 * C + 4], f16)
            wt_mid = consts.tile([C, 4 * C], f16)

            def wslice(i):
                if i < 2:
                    return wt_lo[:, i * C : (i + 1) * C]
                return wt_mid[:, (i - 2) * C : (i - 1) * C]

            def bias(j):  # 0: bz, 1: br, 2: bh, 3: -bz
                return wt_lo[:, 2 * C + j : 2 * C + j + 1]

            def load_x(t):
                xt = xpool.tile([C, P], f16, tag="x")
                nc.sync.dma_start(xt[:], x_ap[t])
                return xt

            # startup DMA, ordered by first use.  sync (HWDGE): group-a
            # x0 half, t1 weights, x1b, x2.  gpsimd (SWDGE): t0 weights
            # (+biases), group-b x0 half, x1a.  The scalar queue carries
            # no startup DMA: a dispatch next to the warmup sigmoid makes
            # the compiler emit a second 1283ns ACT_TABLE_LOAD.
            x0h = [xpool.tile([C, PG], f16, tag="x0a", name="x0a"),
                   xpool.tile([C, PG], f16, tag="x0b", name="x0b")]
            x1h = [xpool.tile([C, PG], f16, tag="x1a", name="x1a"),
                   xpool.tile([C, PG], f16, tag="x1b", name="x1b")]
            nc.sync.dma_start(x0h[0][:], x_ap[0, :, :PG])
            nc.gpsimd.dma_start(wt_lo[:], w_ap[:, : 2 * C + 4])
            nc.gpsimd.dma_start(x0h[1][:], x_ap[0, :, PG:])
            nc.sync.dma_start(wt_mid[:], w_ap[:, 2 * C + 4 :])
            nc.sync.dma_start(x1h[0][:], x_ap[1, :, :PG])
            nc.gpsimd.dma_start(x1h[1][:], x_ap[1, :, PG:])
            x_tiles = {}
            x_tiles[2] = load_x(2)

            # -- warmup: ramp the PE clock gate with matmuls that only
            #    depend on a memset tile, while the input DMAs fly --
            warm16 = wk.tile([C, PG], f16, tag="warm")
            nc.vector.memset(warm16[:], 0.0)
            cwarm = [None, None]
            for g in range(G):
                cwarm[g] = ps.tile(
                    [C, PG], f32, tag=f"c_{g}", bufs=2, name=f"cwarm_{g}"
                )
            nc.tensor.matmul(
                cwarm[0][:], warm16[:, :C], warm16[:],
                start=True, stop=True,
            )
            # preload the ACT sigmoid/tanh table right after the first
            # warmup matmul; const bias so it has no DMA dependency
            wtmp = wk.tile([C, PG], f16, tag="scratch")
            nc.scalar.activation(wtmp[:], cwarm[0][:], AF.Sigmoid)
            for i in range(1, 4):
                nc.tensor.matmul(
                    cwarm[i % 2][:], warm16[:, :C], warm16[:],
                    start=True, stop=True,
                )

            def open_zr(xs, g):
                """Open one group's z|r accumulation with the x-side."""
                zrt = ps.tile([C, 2 * PG], f32, tag=f"zr_{g}", bufs=1,
                              name=f"zr_t{g}")
                nc.tensor.matmul(
                    zrt[:, PG:], wslice(WRX), xs, start=True, stop=False
                )
                nc.tensor.matmul(
                    zrt[:, :PG], wslice(WZX), xs, start=True, stop=False
                )
                return zrt

            def open_c(xs, g):
                cp = ps.tile([C, PG], f32, tag=f"c_{g}", bufs=2,
                             name=f"c_t{g}")
                nc.tensor.matmul(
                    cp[:], wslice(WHX), xs, start=True, stop=False
                )
                return cp

            # ---- t = 0: h0 == 0, so no closers / r-gate / rh ----
            zr0 = [None, None]
            c0 = [None, None]
            for g in range(G):
                zrt = ps.tile([C, 2 * PG], f32, tag=f"zr_{g}", name=f"zr0_{g}")
                nc.tensor.matmul(
                    zrt[:, :PG], wslice(WZX), x0h[g][:],
                    start=True, stop=True,
                )
                zr0[g] = zrt
                cp = ps.tile([C, PG], f32, tag=f"c_{g}", bufs=2)
                nc.tensor.matmul(
                    cp[:], wslice(WHX), x0h[g][:],
                    start=True, stop=True,
                )
                c0[g] = cp
            h16 = [None, None]
            for g in range(G):
                zt = wk.tile([C, PG], f16, tag=f"zb_{g}")
                nc.scalar.activation(
                    zt[:], zr0[g][:, :PG], AF.Sigmoid, bias=bias(0)
                )
                ct = wk.tile([C, PG], f16, tag=f"c16_{g}")
                nc.scalar.activation(ct[:], c0[g][:], AF.Tanh, bias=bias(2))
                ht = spool.tile([C, PG], f16, tag=f"h16_{g}")
                nc.vector.tensor_mul(ht[:], zt[:], ct[:])
                h16[g] = ht
                nc.sync.dma_start(
                    o_ap[0, :, g * PG : (g + 1) * PG], ht[:]
                )

            # open + close t=1's zr with h'(0) (plain single closes)
            zr_t = [None, None]
            cp_t = [None, None]
            for g in range(G):
                zr_t[g] = open_zr(x1h[g][:], g)
                nc.tensor.matmul(
                    zr_t[g][:, PG:], wslice(WRH), h16[g][:],
                    start=False, stop=True,
                )
                nc.tensor.matmul(
                    zr_t[g][:, :PG], wslice(WZH), h16[g][:],
                    start=False, stop=True,
                )
                cp_t[g] = open_c(x1h[g][:], g)

            # ---- steady steps t = 1..T-1 ----
            # zr_t arrives FULLY CLOSED (r closed via Wrh@u + Wrh@v of the
            # previous step's blend; z closed via Wzh@h')
            for t in range(1, T):
                go = [0, 1] if t % 2 == 1 else [1, 0]
                a, b = go
                x_next = x_tiles.get(t + 1)
                if t + 2 < T:
                    x_tiles[t + 2] = load_x(t + 2)

                r16, zb16 = [None] * G, [None] * G

                def sig_r(g):
                    rt = wk.tile([C, PG], f16, tag=f"r_{g}", name=f"r16_{g}")
                    nc.scalar.activation(
                        rt[:], zr_t[g][:, PG:], AF.Sigmoid, bias=bias(1)
                    )
                    r16[g] = rt

                def sig_zbar(g):
                    zbt = wk.tile([C, PG], f16, tag=f"zb_{g}", name=f"zb16_{g}")
                    nc.scalar.activation(
                        zbt[:], zr_t[g][:, :PG], AF.Sigmoid,
                        bias=bias(3), scale=-1.0,
                    )
                    zb16[g] = zbt

                sig_r(a)
                sig_zbar(a)
                sig_r(b)

                # group a's next-step z|r openers (zr_a fully consumed)
                zr_next = [None] * G
                if x_next is not None:
                    zr_next[a] = open_zr(x_next[:, a * PG : (a + 1) * PG], a)

                rh16 = [None] * G
                for g in go:
                    rh = wk.tile([C, PG], f16, tag=f"rh_{g}")
                    nc.vector.tensor_mul(rh[:], r16[g][:], h16[g][:])
                    rh16[g] = rh

                for g in go:
                    nc.tensor.matmul(
                        cp_t[g][:], wslice(WHH), rh16[g][:],
                        start=False, stop=True,
                    )

                cp_next = [None] * G

                c16 = [None] * G

                def tanh_c(g):
                    ct = wk.tile([C, PG], f16, tag=f"c16_{g}", name=f"c16_{g}")
                    nc.scalar.activation(
                        ct[:], cp_t[g][:], AF.Tanh, bias=bias(2)
                    )
                    c16[g] = ct

                u16, z16 = [None] * G, [None] * G

                def blend_pre(g):
                    """u,z depend only on zbar; on the last step they are
                    hoisted before the tanh so the final h'-adds start the
                    moment the tanh lands."""
                    ut = wk.tile([C, PG], f16, tag=f"u_{g}", name=f"u16_{g}")
                    nc.vector.tensor_mul(ut[:], zb16[g][:], h16[g][:])
                    zt = wk.tile([C, PG], f16, tag=f"z_{g}", name=f"z16_{g}")
                    nc.vector.tensor_scalar(
                        zt[:], zb16[g][:], -1.0, 1.0,
                        mybir.AluOpType.mult, mybir.AluOpType.add,
                    )
                    u16[g], z16[g] = ut, zt
                    if zr_next[g] is not None:
                        nc.tensor.matmul(
                            zr_next[g][:, PG:], wslice(WRH), ut[:],
                            start=False, stop=False,
                        )

                def blend(g):
                    """v + h'-add after tanh; the next step's r-close rides
                    u and v so the sigmoid path never waits for the add."""
                    ut, zt = u16[g], z16[g]
                    v16 = wk.tile([C, PG], f16, tag=f"v_{g}", name=f"v16_{g}")
                    nc.vector.tensor_mul(v16[:], zt[:], c16[g][:])
                    if zr_next[g] is not None:
                        with tc.high_priority(offset=25):
                            nc.tensor.matmul(
                                zr_next[g][:, PG:], wslice(WRH), v16[:],
                                start=False, stop=True,
                            )
                    n16 = spool.tile([C, PG], f16, tag=f"h16_{g}",
                                     name=f"h16n_{g}")
                    if t + 1 < T:
                        nc.vector.tensor_add(n16[:], ut[:], v16[:])
                        h16[g] = n16
                        nc.tensor.matmul(
                            zr_next[g][:, :PG], wslice(WZH), n16[:],
                            start=False, stop=True,
                        )
                        nc.gpsimd.dma_start(
                            o_ap[t, :, g * PG : (g + 1) * PG], n16[:]
                        )
                    else:
                        # final step: chunk the add so each half's store
                        # dispatches as soon as that half is summed
                        hp = PG // 2
                        nc.vector.tensor_add(
                            n16[:, :hp], ut[:, :hp], v16[:, :hp]
                        )
                        nc.scalar.dma_start(
                            o_ap[t, :, g * PG : g * PG + hp], n16[:, :hp]
                        )
                        nc.vector.tensor_add(
                            n16[:, hp:], ut[:, hp:], v16[:, hp:]
                        )
                        nc.sync.dma_start(
                            o_ap[t, :, g * PG + hp : (g + 1) * PG], n16[:, hp:]
                        )
                        h16[g] = n16

                last = x_next is None
                if last:
                    blend_pre(a)
                tanh_c(a)
                sig_zbar(b)
                if not last:
                    blend_pre(a)
                blend(a)
                if not last:
                    for g in go:
                        cp_next[g] = open_c(x_next[:, g * PG : (g + 1) * PG], g)
                if last:
                    blend_pre(b)
                tanh_c(b)
                if not last:
                    zr_next[b] = open_zr(x_next[:, b * PG : (b + 1) * PG], b)
                    blend_pre(b)
                blend(b)

                x_tiles.pop(t - 1, None)
                if x_next is not None:
                    zr_t, cp_t = zr_next, cp_next

    nc.compile()
    return nc


def _get_prog():
    global _PROG
    if _PROG is None:
        _PROG = _build()
    return _PROG


def _make_in_maps(video, Wz, bz, Wr, br, Wh, bh):
    b4 = np.stack([bz, br, bh, -bz], axis=1)
    w6 = np.concatenate(
        [
            Wz[:, :C].T, Wh[:, :C].T, b4, Wr[:, :C].T,
            Wz[:, C:].T, Wr[:, C:].T, Wh[:, C:].T,
        ],
        axis=1,
    ).astype(np.float16)
    in_maps = []
    for core in range(NCORES):
        b_, q = divmod(core, 4)
        xs = np.ascontiguousarray(
            video[b_, :, :, q * HQ : (q + 1) * HQ, :]
        ).reshape(T, C, P).astype(np.float16)
        in_maps.append({"x_seq": xs, "wmats": w6})
    return in_maps


def kernel(video, Wz, bz, Wr, br, Wh, bh):
    _ensure_paths()
    from concourse.bass_utils import run_bass_kernel_spmd

    video = np.asarray(video, dtype=np.float32)
    nc = _get_prog()
    in_maps = _make_in_maps(video, Wz, bz, Wr, br, Wh, bh)
    res = run_bass_kernel_spmd(nc, in_maps, list(range(NCORES)))

    out = np.empty((B, T, C, H, W), np.float32)
    for core in range(NCORES):
        b_, q = divmod(core, 4)
        out[b_, :, :, q * HQ : (q + 1) * HQ, :] = np.asarray(
            res.results[core]["out_seq"]
        ).astype(np.float32).reshape(T, C, HQ, W)
    return out



# revision 21
# speedup vs baseline: 22306.3540x; 22306.3540x over previous
"""ConvGRU Trainium2 kernel.

video [B=2, T=16, C=128, H=64, W=64] f32; 1x1-conv GRU over T.
Sharding: data-parallel over (B x H/16) -> 8 cores, each core owns
P = 16*64 = 1024 pixels for all T; weights replicated.

Per core, per timestep (pixels on the free dim, channels on partitions):
    zr_pre = [Wzx@x + Wzh@h | Wrx@x + Wrh@h]      (PE, fp16 in / fp32 psum)
    z = sigmoid(zr_pre[:P] + bz); r = sigmoid(zr_pre[P:] + br)   (ACT)
    rh = r * h                                     (DVE)
    c = tanh(Whx@x + Whh@rh + bh)                  (PE + ACT)
    h' = u + v,  u = zbar*h,  v = z*c,  zbar = sigmoid(-pre_z)

G=2 pixel groups form two independent recurrence chains that
interleave on the engines.  The Scalar (ACT) engine is the pacing
resource: 6 sigmoid/tanh ops x ~690ns = 4.13us/step of streaming.
The remaining slack is the serial tail between the last tanh and the
next step's first r-sigmoid.  Structure choices that close it:

  - The next step's r-gate close is DISTRIBUTED over h' = u + v:
        pre_r(t+1) += Wrh@u(t)   (issues mid-step, u is ready early)
        pre_r(t+1) += Wrh@v(t)   (right after v -- the h' add leaves
                                  the sigmoid critical path entirely)
    The z-gate close stays a single Wzh@h' (zbar sits early in the
    next step's ACT stream, so it has slack).
  - Each group's zbar runs immediately after its own r-sigmoid, so
    that group's next-step zr openers (WAR on the single-buffered zr
    PSUM tile) clear the PE FIFO long before the r-closes arrive.
  - DVE tail is group-major (u,z,v,add per group): the first group's
    v/add never queue behind the second group's u/z, whose zbar lands
    later on the ACT stream.
  - t=0 shortcut: h0 == 0, so closers, r-sigmoid and rh are skipped.
  - fp16 everywhere: bf16 measures uniformly slower on this stack
    (ACTIVATE 687->823ns, TT 423->508ns); fp16 matmuls already
    pipeline at the 216ns/MM N=512 roofline.
  - DMA traffic is split across the two fast queues (sync HWDGE +
    gpsimd SWDGE): per step one x prefetch (t+2 ahead) rides sync and
    the two h' output stores ride gpsimd.  The last step's outputs
    split across the scalar+sync queues instead (the ACT queue is
    idle by then and the gpsimd dge_drain would add ~3.6us of tail),
    with the final h'-adds chunked so each half's store dispatches as
    soon as that half is summed.
  - Startup: weights live in TWO tiles (Wzx|Whx|biases vs the rest)
    and x0/x1 in per-group half tiles, so t0's matmuls gate only on
    their own DMAs (dependency tracking is tile-granular).  Biases
    ride as 4 fp16 columns inside the first weight DMA (a separate
    [C,4] f32 tensor costs ~0.9us of 16-byte-packet DMA).  Queues:
    sync carries x0a/w-mid/x1a/x2, gpsimd carries w-lo/x0b/x1b, and
    the scalar queue carries NOTHING early -- a scalar dispatch next
    to the warmup sigmoid makes the compiler emit a second 1283ns
    ACT_TABLE_LOAD.
  - PE warmup matmuls run against a memset tile (no weight-DMA
    dependency); the warmup sigmoid follows the FIRST of them so the
    activation-table load finishes during the DMA ramp.  Ramp-phase
    matmuls run ~3x slow (pstate ramp), which keeps t0/t1 at ~10.5
    and ~15.9us despite data landing by ~12us.

Measured: 84.3-84.7us (3-run spread) vs 85.7-86.3us for the prior
best; steady-state period ~4210ns/step vs the 4128ns ACT-streaming
floor.  Run-to-run the shared-chip clock varies up to ~1.2x, so
single measurements of this kernel family overlap; structure is
compared via clock-normalized traces (ACTIVATE=687ns at full clock).

Numerics: fp16 matmul inputs/gates/state/biases, fp32 PSUM accum.
Failed directions (measured): merging z|r sigmoids into [C,1024]
ACTIVATEs needs per-half PSUM bias pre-fills that no engine can
afford (DVE tensor ops are 424ns, fills 833-1467ns, and GPSIMD has
no PSUM port); DVE at 3.96us/step secretly co-paces with ACT at
4.13, so any design that adds DVE work loses.
"""

import os
import sys

import numpy as np

B, T, C, H, W = 2, 16, 128, 64, 64
NCORES = 8
HQ = H // 4          # 16 rows of H per core (4 H-slices x 2 batches = 8 cores)
P = HQ * W           # 1024 pixels per core
G = 2                # pixel groups per step (independent recurrence chains)
PG = P // G          # 512 pixels per group

_PROG = None


def _ensure_paths():
    for p in ("/opt/trn_rl_repo",):
        if p not in sys.path and os.path.isdir(p):
            sys.path.append(p)


def _build():
    _ensure_paths()
    import concourse.bacc as bacc
    import concourse.tile as tile
    from concourse import mybir

    f32 = mybir.dt.float32
    f16 = mybir.dt.float16
    AF = mybir.ActivationFunctionType

    nc = bacc.Bacc(
        "TRN2", target_bir_lowering=False, debug=False, num_devices=NCORES
    )
    x_dram = nc.dram_tensor("x_seq", [T, C, P], f16, kind="ExternalInput")
    # wmats = [Wzx.T | Whx.T | bz,br,bh,-bz | Wrx.T | Wzh.T | Wrh.T | Whh.T]
    # biases ride as 4 fp16 columns inside the first weight DMA -- a
    # separate [C,4] tensor costs ~0.9us of 16B-packet DMA at startup
    w_dram = nc.dram_tensor("wmats", [C, 6 * C + 4], f16, kind="ExternalInput")
    o_dram = nc.dram_tensor("out_seq", [T, C, P], f16, kind="ExternalOutput")

    x_ap = x_dram.ap()
    w_ap = w_dram.ap()
    o_ap = o_dram.ap()

    # weight order in wmats: x-side first so its DMA can land first
    WZX, WHX, WRX, WZH, WRH, WHH = range(6)

    with tile.TileContext(nc) as tc:
        with (
            tc.tile_pool(name="consts", bufs=1) as consts,
            tc.tile_pool(name="xin", bufs=4) as xpool,
            tc.tile_pool(name="state", bufs=4) as spool,
            tc.tile_pool(name="work", bufs=3) as wk,
            tc.tile_pool(name="ps", bufs=1, space="PSUM") as ps,
        ):
            # weights live in two tiles so t0's matmuls only gate on the
            # (Wzx|Whx|biases) DMA, not on the whole weight load; likewise
            # x0/x1 are per-group half tiles so each group's openers gate
            # on their own half.
            wt_lo = consts.tile([C, 2 * C + 4], f16)
            wt_mid = consts.tile([C, 4 * C], f16)

            def wslice(i):
                if i < 2:
                    return wt_lo[:, i * C : (i + 1) * C]
                return wt_mid[:, (i - 2) * C : (i - 1) * C]

            def bias(j):  # 0: bz, 1: br, 2: bh, 3: -bz
                return wt_lo[:, 2 * C + j : 2 * C + j + 1]

            def load_x(t):
                xt = xpool.tile([C, P], f16, tag="x")
                nc.sync.dma_start(xt[:], x_ap[t])
                return xt

            # startup DMA, ordered by first use.  sync (HWDGE): group-a
            # x0 half, t1 weights, x1b, x2.  gpsimd (SWDGE): t0 weights
            # (+biases), group-b x0 half, x1a.  The scalar queue carries
            # no startup DMA: a dispatch next to the warmup sigmoid makes
            # the compiler emit a second 1283ns ACT_TABLE_LOAD.
            x0h = [xpool.tile([C, PG], f16, tag="x0a", name="x0a"),
                   xpool.tile([C, PG], f16, tag="x0b", name="x0b")]
            x1h = [xpool.tile([C, PG], f16, tag="x1a", name="x1a"),
                   xpool.tile([C, PG], f16, tag="x1b", name="x1b")]
            nc.sync.dma_start(x0h[0][:], x_ap[0, :, :PG])
            nc.gpsimd.dma_start(wt_lo[:], w_ap[:, : 2 * C + 4])
            nc.gpsimd.dma_start(x0h[1][:], x_ap[0, :, PG:])
            nc.sync.dma_start(wt_mid[:], w_ap[:, 2 * C + 4 :])
            nc.sync.dma_start(x1h[0][:], x_ap[1, :, :PG])
            nc.gpsimd.dma_start(x1h[1][:], x_ap[1, :, PG:])
            x_tiles = {}
            x_tiles[2] = load_x(2)

            # -- warmup: ramp the PE clock gate with matmuls that only
            #    depend on a memset tile, while the input DMAs fly --
            warm16 = wk.tile([C, PG], f16, tag="warm")
            nc.vector.memset(warm16[:], 0.0)
            cwarm = [None, None]
            for g in range(G):
                cwarm[g] = ps.tile(
                    [C, PG], f32, tag=f"c_{g}", bufs=2, name=f"cwarm_{g}"
                )
            nc.tensor.matmul(
                cwarm[0][:], warm16[:, :C], warm16[:],
                start=True, stop=True,
            )
            # preload the ACT sigmoid/tanh table right after the first
            # warmup matmul; const bias so it has no DMA dependency
            wtmp = wk.tile([C, PG], f16, tag="scratch")
            nc.scalar.activation(wtmp[:], cwarm[0][:], AF.Sigmoid)
            for i in range(1, 4):
                nc.tensor.matmul(
                    cwarm[i % 2][:], warm16[:, :C], warm16[:],
                    start=True, stop=True,
                )

            def open_zr(xs, g):
                """Open one group's z|r accumulation with the x-side."""
                zrt = ps.tile([C, 2 * PG], f32, tag=f"zr_{g}", bufs=1,
                              name=f"zr_t{g}")
                nc.tensor.matmul(
                    zrt[:, PG:], wslice(WRX), xs, start=True, stop=False
                )
                nc.tensor.matmul(
                    zrt[:, :PG], wslice(WZX), xs, start=True, stop=False
                )
                return zrt

            def open_c(xs, g):
                cp = ps.tile([C, PG], f32, tag=f"c_{g}", bufs=2,
                             name=f"c_t{g}")
                nc.tensor.matmul(
                    cp[:], wslice(WHX), xs, start=True, stop=False
                )
                return cp

            # ---- t = 0: h0 == 0, so no closers / r-gate / rh ----
            zr0 = [None, None]
            c0 = [None, None]
            for g in range(G):
                zrt = ps.tile([C, 2 * PG], f32, tag=f"zr_{g}", name=f"zr0_{g}")
                nc.tensor.matmul(
                    zrt[:, :PG], wslice(WZX), x0h[g][:],
                    start=True, stop=True,
                )
                zr0[g] = zrt
                cp = ps.tile([C, PG], f32, tag=f"c_{g}", bufs=2)
                nc.tensor.matmul(
                    cp[:], wslice(WHX), x0h[g][:],
                    start=True, stop=True,
                )
                c0[g] = cp
            h16 = [None, None]
            for g in range(G):
                zt = wk.tile([C, PG], f16, tag=f"zb_{g}")
                nc.scalar.activation(
                    zt[:], zr0[g][:, :PG], AF.Sigmoid, bias=bias(0)
                )
                ct = wk.tile([C, PG], f16, tag=f"c16_{g}")
                nc.scalar.activation(ct[:], c0[g][:], AF.Tanh, bias=bias(2))
                ht = spool.tile([C, PG], f16, tag=f"h16_{g}")
                nc.vector.tensor_mul(ht[:], zt[:], ct[:])
                h16[g] = ht
                nc.gpsimd.dma_start(
                    o_ap[0, :, g * PG : (g + 1) * PG], ht[:]
                )

            # open + close t=1's zr with h'(0) (plain single closes)
            zr_t = [None, None]
            cp_t = [None, None]
            for g in range(G):
                # r-half open+close paired first: t=1's r-sigmoid gates on
                # these two matmuls only (the z-half has a 687ns slot of
                # slack behind it on the ACT stream)
                zrt = ps.tile([C, 2 * PG], f32, tag=f"zr_{g}", bufs=1,
                              name=f"zr_t{g}")
                nc.tensor.matmul(
                    zrt[:, PG:], wslice(WRX), x1h[g][:],
                    start=True, stop=False,
                )
                nc.tensor.matmul(
                    zrt[:, PG:], wslice(WRH), h16[g][:],
                    start=False, stop=True,
                )
                nc.tensor.matmul(
                    zrt[:, :PG], wslice(WZX), x1h[g][:],
                    start=True, stop=False,
                )
                nc.tensor.matmul(
                    zrt[:, :PG], wslice(WZH), h16[g][:],
                    start=False, stop=True,
                )
                zr_t[g] = zrt
                cp_t[g] = open_c(x1h[g][:], g)

            # ---- steady steps t = 1..T-1 ----
            # zr_t arrives FULLY CLOSED (r closed via Wrh@u + Wrh@v of the
            # previous step's blend; z closed via Wzh@h')
            for t in range(1, T):
                go = [0, 1] if t % 2 == 1 else [1, 0]
                a, b = go
                x_next = x_tiles.get(t + 1)
                if t + 2 < T:
                    x_tiles[t + 2] = load_x(t + 2)

                r16, zb16 = [None] * G, [None] * G

                def sig_r(g):
                    rt = wk.tile([C, PG], f16, tag=f"r_{g}", name=f"r16_{g}")
                    nc.scalar.activation(
                        rt[:], zr_t[g][:, PG:], AF.Sigmoid, bias=bias(1)
                    )
                    r16[g] = rt

                def sig_zbar(g):
                    zbt = wk.tile([C, PG], f16, tag=f"zb_{g}", name=f"zb16_{g}")
                    nc.scalar.activation(
                        zbt[:], zr_t[g][:, :PG], AF.Sigmoid,
                        bias=bias(3), scale=-1.0,
                    )
                    zb16[g] = zbt

                sig_r(a)
                sig_zbar(a)
                sig_r(b)

                # group a's next-step z|r openers (zr_a fully consumed)
                zr_next = [None] * G
                if x_next is not None:
                    zr_next[a] = open_zr(x_next[:, a * PG : (a + 1) * PG], a)

                rh16 = [None] * G
                for g in go:
                    rh = wk.tile([C, PG], f16, tag=f"rh_{g}")
                    nc.vector.tensor_mul(rh[:], r16[g][:], h16[g][:])
                    rh16[g] = rh

                for g in go:
                    nc.tensor.matmul(
                        cp_t[g][:], wslice(WHH), rh16[g][:],
                        start=False, stop=True,
                    )

                cp_next = [None] * G

                c16 = [None] * G

                def tanh_c(g):
                    ct = wk.tile([C, PG], f16, tag=f"c16_{g}", name=f"c16_{g}")
                    nc.scalar.activation(
                        ct[:], cp_t[g][:], AF.Tanh, bias=bias(2)
                    )
                    c16[g] = ct

                u16, z16 = [None] * G, [None] * G

                def blend_pre(g):
                    """u,z depend only on zbar; on the last step they are
                    hoisted before the tanh so the final h'-adds start the
                    moment the tanh lands."""
                    ut = wk.tile([C, PG], f16, tag=f"u_{g}", name=f"u16_{g}")
                    nc.vector.tensor_mul(ut[:], zb16[g][:], h16[g][:])
                    zt = wk.tile([C, PG], f16, tag=f"z_{g}", name=f"z16_{g}")
                    nc.vector.tensor_scalar(
                        zt[:], zb16[g][:], -1.0, 1.0,
                        mybir.AluOpType.mult, mybir.AluOpType.add,
                    )
                    u16[g], z16[g] = ut, zt
                    if zr_next[g] is not None:
                        nc.tensor.matmul(
                            zr_next[g][:, PG:], wslice(WRH), ut[:],
                            start=False, stop=False,
                        )

                def blend(g):
                    """v + h'-add after tanh; the next step's r-close rides
                    u and v so the sigmoid path never waits for the add."""
                    ut, zt = u16[g], z16[g]
                    v16 = wk.tile([C, PG], f16, tag=f"v_{g}", name=f"v16_{g}")
                    nc.vector.tensor_mul(v16[:], zt[:], c16[g][:])
                    if zr_next[g] is not None:
                        with tc.high_priority(offset=25):
                            nc.tensor.matmul(
                                zr_next[g][:, PG:], wslice(WRH), v16[:],
                                start=False, stop=True,
                            )
                    n16 = spool.tile([C, PG], f16, tag=f"h16_{g}",
                                     name=f"h16n_{g}")
                    if t + 1 < T:
                        nc.vector.tensor_add(n16[:], ut[:], v16[:])
                        h16[g] = n16
                        nc.tensor.matmul(
                            zr_next[g][:, :PG], wslice(WZH), n16[:],
                            start=False, stop=True,
                        )
                        nc.gpsimd.dma_start(
                            o_ap[t, :, g * PG : (g + 1) * PG], n16[:]
                        )
                    else:
                        # final step: chunk the add so each half's store
                        # dispatches as soon as that half is summed
                        hp = PG // 2
                        nc.vector.tensor_add(
                            n16[:, :hp], ut[:, :hp], v16[:, :hp]
                        )
                        nc.scalar.dma_start(
                            o_ap[t, :, g * PG : g * PG + hp], n16[:, :hp]
                        )
                        nc.vector.tensor_add(
                            n16[:, hp:], ut[:, hp:], v16[:, hp:]
                        )
                        nc.sync.dma_start(
                            o_ap[t, :, g * PG + hp : (g + 1) * PG], n16[:, hp:]
                        )
                        h16[g] = n16

                last = x_next is None
                if last:
                    blend_pre(a)
                tanh_c(a)
                sig_zbar(b)
                if not last:
                    blend_pre(a)
                blend(a)
                if not last:
                    for g in go:
                        cp_next[g] = open_c(x_next[:, g * PG : (g + 1) * PG], g)
                if last:
                    blend_pre(b)
                tanh_c(b)
                if not last:
                    zr_next[b] = open_zr(x_next[:, b * PG : (b + 1) * PG], b)
                    blend_pre(b)
                blend(b)

                x_tiles.pop(t - 1, None)
                if x_next is not None:
                    zr_t, cp_t = zr_next, cp_next

    nc.compile()
    return nc


def _get_prog():
    global _PROG
    if _PROG is None:
        _PROG = _build()
    return _PROG


def _make_in_maps(video, Wz, bz, Wr, br, Wh, bh):
    b4 = np.stack([bz, br, bh, -bz], axis=1)
    w6 = np.concatenate(
        [
            Wz[:, :C].T, Wh[:, :C].T, b4, Wr[:, :C].T,
            Wz[:, C:].T, Wr[:, C:].T, Wh[:, C:].T,
        ],
        axis=1,
    ).astype(np.float16)
    in_maps = []
    for core in range(NCORES):
        b_, q = divmod(core, 4)
        xs = np.ascontiguousarray(
            video[b_, :, :, q * HQ : (q + 1) * HQ, :]
        ).reshape(T, C, P).astype(np.float16)
        in_maps.append({"x_seq": xs, "wmats": w6})
    return in_maps


def kernel(video, Wz, bz, Wr, br, Wh, bh):
    _ensure_paths()
    from concourse.bass_utils import run_bass_kernel_spmd

    video = np.asarray(video, dtype=np.float32)
    nc = _get_prog()
    in_maps = _make_in_maps(video, Wz, bz, Wr, br, Wh, bh)
    res = run_bass_kernel_spmd(nc, in_maps, list(range(NCORES)))

    out = np.empty((B, T, C, H, W), np.float32)
    for core in range(NCORES):
        b_, q = divmod(core, 4)
        out[b_, :, :, q * HQ : (q + 1) * HQ, :] = np.asarray(
            res.results[core]["out_seq"]
        ).astype(np.float32).reshape(T, C, HQ, W)
    return out

